# revision 27
# baseline (speedup 1.0000x reference)
"""DiceLoss kernel for Trainium2 (raw Bass, no Tile), 8-core data parallel.

Problem: predict/target [2, 4, 64, 256, 256] f32.
  p = sigmoid(predict); per (b, o, d) slice of 65536 elements:
    num = sum(p*t), den = sum(p) + sum(t) + 1
    dice = 1 - 2*num/den
  per-(b,o) mean over valid d slices, then mean over the 8 (b,o) pairs.

Sharding: B*O = 8 (b, o) pairs, one per core; each core handles its
pair's 64 depth slices (16 MiB predict + 16 MiB target -> DMA-bound,
~78 us/core measured, ~429 GB/s effective). Host interleaves predict and
target per slice into one [64, 2, 128, 512] array per core so a load
step is a single dma_start (4 MiB for full 8-slice groups).

Per slice (viewed [128 x 512]) on each core:
  ScalarE: sigmoid(predict) -> sig, accum_out -> sum(p) column
  VectorE: scalar_tensor_tensor (sig*1.0)*target -> scratch,
           accum_out -> sum(p*t) column   (single fused op)
  sum(t):  split across the two engines so neither exceeds the DMA
           floor — odd slices on ScalarE (Copy + accum_out; Copy and
           Sigmoid share one ACT table -> no reloads), even slices on
           VectorE (tensor_scalar *1.0 + accum_out).
(tensor_tensor_reduce is not used: its InstISA encoding is rejected by
this walrus build. Tile is not used: its kernel-tail drain exceeds this
build's per-instruction sync-wait limit.)

Scratch outputs rotate over 2 slots so same-engine WAW conflicts sit at
distance 2; a wait_ge on the engine's own (already-passed) semaphore
value proves the ordering to the race detector without stalling.

The [128, 3, 64] accumulator tile is DMA'd out once; host does the
partition sum and the tiny dice math over 512 slices.
"""

from contextlib import ExitStack

import numpy as np

import concourse.bass as bass
from concourse import mybir
from concourse.bass_utils import run_bass_kernel_spmd

N_CORES = 8
B, O, D = 2, 4, 64
HW = 256 * 256          # elements per slice
P = 128                 # SBUF partitions
F = HW // P             # 512 free elems per partition per slice
S = (B * O * D) // N_CORES  # 64 slices per core (= one (b,o) pair)
G = 8                   # slices per full DMA group (4 MiB per dma_start)
# The last TAIL_STEPS*TAIL_CNT slices load as small steps into a DEDICATED
# buffer with per-step semaphores (no recycle coupling with the rotating
# slots), so the after-last-byte compute tail shrinks from ~1 full group
# (~7 us) to ~TAIL_CNT slices (~2 us).
TAIL_STEPS = 4
TAIL_CNT = 2
IN_BUFS = 3             # input step buffers
SIG_BUFS = 2            # sigmoid output step buffers
SMOOTH = 1.0

f32 = mybir.dt.float32
AF = mybir.ActivationFunctionType
ALU = mybir.AluOpType


def _make_steps(n_slices, group, tail_steps, tail_cnt):
    """Load schedule: (start_slice, count, is_tail) steps — full groups,
    then small trailing steps to shrink the end-of-kernel compute tail."""
    tail = tail_steps * tail_cnt
    if n_slices <= tail or tail == 0:
        tail = 0
    steps = []
    pos = 0
    while pos < n_slices - tail:
        c = min(group, n_slices - tail - pos)
        steps.append((pos, c, False))
        pos += c
    while pos < n_slices:
        c = min(tail_cnt, n_slices - pos)
        steps.append((pos, c, True))
        pos += c
    return steps


def build_nc(n_slices=S, group=G, repeats=1, tail_steps=TAIL_STEPS,
             tail_cnt=TAIL_CNT):
    """Build the per-core Bass program (same program on all cores).

    repeats > 1 re-runs the whole body that many times (re-reading the
    same DRAM) — used only for slope-based wall-clock timing."""
    per_pass = _make_steps(n_slices, group, tail_steps, tail_cnt)
    n_tail = sum(1 for _, _, it in per_pass if it)
    steps = []
    for r in range(repeats):
        k = 0
        for (st, cnt, is_tail) in per_pass:
            steps.append((st, cnt, is_tail, k if is_tail else None))
            if is_tail:
                k += 1
    n_steps = len(steps)
    # prefix sums: slice_base[i] = slices completed before step i,
    # copy_base[i] = ACT copies completed before step i
    slice_base = [0]
    copy_base = [0]
    for _, cnt, _, _ in steps:
        slice_base.append(slice_base[-1] + cnt)
        copy_base.append(copy_base[-1] + cnt // 2)
    total_slices = slice_base[-1]
    total_copies = copy_base[-1]

    # slot assignment + per-slot use counts + recycle source
    slot_of = []        # per step: ("b", idx) or ("t", idx)
    use_of = []         # per step: 1-based use count of its slot/sem
    prev_user = []      # per step: previous step index using the slot, or None
    last_user = {}
    uses = {}
    bi = 0
    for i, (st, cnt, is_tail, k) in enumerate(steps):
        key = ("t", k) if is_tail else ("b", bi % IN_BUFS)
        if not is_tail:
            bi += 1
        slot_of.append(key)
        uses[key] = uses.get(key, 0) + 1
        use_of.append(uses[key])
        prev_user.append(last_user.get(key))
        last_user[key] = i

    nc = bass.Bass("TRN2", debug=False, enable_asserts=False)

    # inp[s, 0] = predict slice s, inp[s, 1] = target slice s (host-stacked);
    # the (s, 2) dims merge into one stride run so a step load is a 3-dim AP.
    inp = nc.dram_tensor("inp", [n_slices, 2, P, F], f32, kind="ExternalInput").ap()
    # out_acc[:, 0] = sum(p), out_acc[:, 1] = sum(t), out_acc[:, 2] = sum(p*t)
    out_acc = nc.dram_tensor("out_acc", [P, 3, n_slices], f32,
                             kind="ExternalOutput").ap()

    with ExitStack() as ctx:
        # in_buf slot layout: m = 2*s + j blocks of F: predict slice s at
        # m=2s, target slice s at m=2s+1 (s local to the step)
        in_buf = ctx.enter_context(nc.sbuf_tensor([P, IN_BUFS, 2 * group * F], f32))
        tail_buf = None
        if n_tail:
            tail_buf = ctx.enter_context(
                nc.sbuf_tensor("tail_buf", [P, n_tail, 2 * tail_cnt * F], f32))
        sig_buf = ctx.enter_context(nc.sbuf_tensor([P, SIG_BUFS, group * F], f32))
        scr_a = ctx.enter_context(nc.sbuf_tensor([P, 2, F], f32))   # ACT copy dest
        scr_v = ctx.enter_context(nc.sbuf_tensor([P, 2, F], f32))   # DVE stt dest
        scr_t = ctx.enter_context(nc.sbuf_tensor([P, 2, F], f32))   # DVE ts dest
        acc = ctx.enter_context(nc.sbuf_tensor([P, 3, n_slices], f32))
        # One DMA sem per buffer slot: at most one load in flight per sem,
        # so "sem >= 16*uses" proves that load is complete. A single
        # cumulative sem would be unsound with >1 DMA in flight (fast SDMA
        # engines can contribute shards of later DMAs to the count).
        dma_sems = {}
        for i in range(IN_BUFS):
            dma_sems[("b", i)] = ctx.enter_context(nc.semaphore(f"dma_b{i}"))
        for kk in range(n_tail):
            dma_sems[("t", kk)] = ctx.enter_context(nc.semaphore(f"dma_t{kk}"))
        out_sem = ctx.enter_context(nc.semaphore("out_sem"))
        sig_sem = ctx.enter_context(nc.semaphore("sig_sem"))    # +1 per sigmoid
        actc_sem = ctx.enter_context(nc.semaphore("actc_sem"))  # +1 per ACT copy
        dve_sem = ctx.enter_context(nc.semaphore("dve_sem"))    # +1 per slice
        block = ctx.enter_context(nc.Block())

        sp_acc = acc[:, 0, :]
        st_acc = acc[:, 1, :]
        spt_acc = acc[:, 2, :]

        def step_buf(i, cnt):
            kind, idx = slot_of[i]
            if kind == "b":
                return in_buf[:, idx, 0:2 * cnt * F]
            return tail_buf[:, idx, 0:2 * cnt * F]

        @block.sync
        def _(sync):
            for i, (st, cnt, is_tail, k) in enumerate(steps):
                j = prev_user[i]
                if j is not None:
                    # consumers of the previous user of this slot are done
                    sync.wait_ge(sig_sem, slice_base[j + 1])
                    sync.wait_ge(actc_sem, copy_base[j + 1])
                    sync.wait_ge(dve_sem, slice_base[j + 1])
                sync.dma_start(
                    step_buf(i, cnt).rearrange("p (m f) -> p m f", f=F),
                    inp[st:st + cnt].rearrange("s j p f -> p (s j) f"),
                ).then_inc(dma_sems[slot_of[i]], 16)
            sync.wait_ge(sig_sem, total_slices)
            sync.wait_ge(actc_sem, total_copies)
            sync.wait_ge(dve_sem, total_slices)
            sync.dma_start(out_acc, acc[:]).then_inc(out_sem, 16)
            sync.wait_ge(out_sem, 16)

        @block.scalar
        def _(scalar):
            cidx = 0  # running ACT-copy counter
            for i, (st, cnt, is_tail, k) in enumerate(steps):
                sslot = i % SIG_BUFS
                buf = step_buf(i, cnt)
                scalar.wait_ge(dma_sems[slot_of[i]], 16 * use_of[i])
                if i >= SIG_BUFS:
                    # DVE must be done reading sig of the step in this sslot
                    scalar.wait_ge(dve_sem, slice_base[i - SIG_BUFS + 1])
                for s in range(cnt):
                    q = st + s               # data/accum column
                    c = slice(s * F, (s + 1) * F)
                    cp = slice((2 * s) * F, (2 * s + 1) * F)      # predict
                    ct = slice((2 * s + 1) * F, (2 * s + 2) * F)  # target
                    nc.scalar.activation(
                        sig_buf[:, sslot, c], buf[:, cp], AF.Sigmoid,
                        accum_out=sp_acc[:, q:q + 1],
                    ).then_inc(sig_sem, 1)
                    if s % 2 == 1:
                        if cidx >= 2:
                            # scr_a slot WAW vs copy cidx-2; already satisfied
                            scalar.wait_ge(actc_sem, cidx - 1)
                        nc.scalar.activation(
                            scr_a[:, cidx % 2, :], buf[:, ct], AF.Copy,
                            accum_out=st_acc[:, q:q + 1],
                        ).then_inc(actc_sem, 1)
                        cidx += 1

        @block.vector
        def _(vector):
            for i, (st, cnt, is_tail, k) in enumerate(steps):
                sslot = i % SIG_BUFS
                buf = step_buf(i, cnt)
                for s in range(cnt):
                    tq = slice_base[i] + s   # absolute slice counter
                    q = st + s
                    c = slice(s * F, (s + 1) * F)
                    ct = slice((2 * s + 1) * F, (2 * s + 2) * F)  # target
                    vector.wait_ge(sig_sem, tq + 1)   # sigmoid(tq) done
                    if tq >= 2:
                        # scr_v/scr_t slot WAW vs ops of slice tq-2;
                        # already satisfied
                        vector.wait_ge(dve_sem, tq - 1)
                    stt = nc.vector.scalar_tensor_tensor(
                        out=scr_v[:, tq % 2, :],
                        in0=sig_buf[:, sslot, c],
                        scalar=1.0,
                        in1=buf[:, ct],
                        op0=ALU.mult, op1=ALU.mult,
                        accum_out=spt_acc[:, q:q + 1],
                    )
                    if s % 2 == 0:
                        nc.vector.tensor_scalar(
                            out=scr_t[:, tq % 2, :], in0=buf[:, ct],
                            scalar1=1.0, scalar2=None,
                            op0=ALU.mult, op1=ALU.add,
                            accum_out=st_acc[:, q:q + 1],
                        ).then_inc(dve_sem, 1)
                    else:
                        stt.then_inc(dve_sem, 1)

    return nc


_NC_CACHE = {}


def _get_nc():
    if "nc" not in _NC_CACHE:
        _NC_CACHE["nc"] = build_nc()
    return _NC_CACHE["nc"]


def shard_inputs(predict, target):
    pred_sh = np.ascontiguousarray(predict, dtype=np.float32).reshape(
        N_CORES, S, P, F)
    tgt_sh = np.ascontiguousarray(target, dtype=np.float32).reshape(
        N_CORES, S, P, F)
    return [
        {"inp": np.stack([pred_sh[i], tgt_sh[i]], axis=1)}
        for i in range(N_CORES)
    ]


def finish(results, target):
    """Host-side: partition-sum [128, 3, S] partials + dice math."""
    sp = np.empty((N_CORES, S), np.float64)
    st = np.empty((N_CORES, S), np.float64)
    spt = np.empty((N_CORES, S), np.float64)
    for i, r in enumerate(results):
        a = r["out_acc"].astype(np.float64).sum(axis=0)   # [3, S]
        sp[i], st[i], spt[i] = a[0], a[1], a[2]

    dice = 1.0 - 2.0 * spt / (sp + st + SMOOTH)          # [B*O, D]
    tfirst = target.reshape(B * O, D, HW)[:, :, 0]       # [B*O, D]
    valid = (tfirst != -1.0).astype(np.float64)
    per_pair = (dice * valid).sum(axis=-1) / valid.sum(axis=-1)  # [B*O]
    return np.array(per_pair.mean(), dtype=np.float32)


def kernel(predict: np.ndarray, target: np.ndarray) -> np.ndarray:
    predict = np.asarray(predict)
    target = np.asarray(target)
    assert predict.shape == (B, O, D, 256, 256)
    in_maps = shard_inputs(predict, target)
    nc = _get_nc()
    res = run_bass_kernel_spmd(nc, in_maps, list(range(N_CORES)))
    return finish(res.results, target)
